# revision 12
# baseline (speedup 1.0000x reference)
"""Trainium2 Bass kernel for nn_DTNNLayer (GNN message passing), 8 NeuronCores.

Strategy:
  - Host-side: permute nodes into 128-node blocks via degree-aware bin-packing
    so every block owns <= EPB in-edges; shard blocks across 8 cores; pad each
    block's edge list to exactly EPB -> identical SPMD program on every core.
  - Device phase A (per block): batched dma_gather of node_h pair-rows
    (transposed into feature-major), per-edge MLPs in feature-major layout
    (weights as stationary lhsT, fp16 matmuls at 1 cy/row), message tanh,
    then segment-sum via onehot-matmul accumulating in PSUM; residual add.
  - One AllGather of fp16 h_new; phase B gathers h_new[src] from the gathered
    table and h_new[dst] from the core-local SBUF table, then the edge EMA
    update, all feature-major.
Outputs are assembled/unpermuted on the host.
"""
import os
import sys
import heapq
import numpy as np

import concourse.bass as bass
import concourse.bacc as bacc
import concourse.mybir as mybir
import concourse.tile as tile
from concourse.masks import make_identity

F32 = mybir.dt.float32
F16 = mybir.dt.float16
I16 = mybir.dt.int16

# full-problem config (matches reference.py setup_inputs)
N_NODES = 50000
N_EDGES = 800000
DN = 64
DE = 32
HID = 128
EMA = 0.8
N_CORES = 8
BPC = 49            # blocks per core
EPB = 2048          # edges per block (padded), multiple of 512
GROUP = 512         # edges per inner group (matmul moving dim)


# ---------------------------------------------------------------- host prep

def _pack_nodes(deg, n_bins, epb):
    """Assign each node to a 128-node bin with degree-sum <= epb.
    Returns perm (perm[slot] = orig node id)."""
    n = len(deg)
    assert n == n_bins * 128
    order = np.argsort(-deg, kind="stable")
    heap = [(0, b) for b in range(n_bins)]
    heapq.heapify(heap)
    counts = np.zeros(n_bins, np.int64)
    sums = np.zeros(n_bins, np.int64)
    assign = np.empty(n, np.int64)
    spill = []
    for v in order:
        d = int(deg[v])
        placed = False
        tmp = []
        while heap:
            s, b = heapq.heappop(heap)
            if counts[b] >= 128:
                continue
            if s + d <= epb:
                assign[v] = b
                counts[b] += 1
                sums[b] += d
                if counts[b] < 128:
                    heapq.heappush(heap, (sums[b], b))
                placed = True
                break
            tmp.append((s, b))
        for item in tmp:
            heapq.heappush(heap, item)
        if not placed:
            spill.append(v)
    if spill:
        raise RuntimeError(f"bin packing failed for {len(spill)} nodes; raise EPB")
    perm = np.empty(n, np.int64)
    pos = np.zeros(n_bins, np.int64)
    for v in range(n):
        b = assign[v]
        perm[b * 128 + pos[b]] = v
        pos[b] += 1
    assert (pos == 128).all()
    return perm


def _wrap_idx(flat):
    """[n] -> [128, n//16] int16 wrapped-16 replicated-8 layout for dma_gather."""
    n = len(flat)
    t = flat.reshape(n // 16, 16).T  # [16, n//16]
    return np.tile(t, (8, 1)).astype(np.int16)


def _prep(inputs, n_cores=N_CORES, bpc=BPC, epb=EPB, n_nodes=N_NODES,
          n_edges=N_EDGES):
    npc = bpc * 128                 # nodes per core
    npad = n_cores * npc
    n_bins = n_cores * bpc
    node_h = np.asarray(inputs["node_h"], np.float32)
    edge_h = np.asarray(inputs["edge_h"], np.float32)
    src = np.asarray(inputs["src"], np.int64)
    dst = np.asarray(inputs["dst"], np.int64)

    deg = np.bincount(dst, minlength=npad)
    perm = _pack_nodes(deg, n_bins, epb)          # perm[slot] = orig (or pad id)
    inv = np.empty(npad, np.int64)
    inv[perm] = np.arange(npad)

    node_h_p = np.zeros((npad, DN), np.float32)
    real = perm < n_nodes
    node_h_p[real] = node_h[perm[real]]

    src_s = inv[src]                              # permuted slots
    dst_s = inv[dst]
    blk = dst_s // 128
    order = np.argsort(blk, kind="stable")        # edges grouped by block
    counts = np.bincount(blk, minlength=n_bins)
    assert counts.max() <= epb, f"block overflow {counts.max()} > {epb}"

    # slot layout: n_bins blocks x epb edge slots
    tot = n_bins * epb
    e_orig = np.full(tot, -1, np.int64)           # original edge id per slot
    starts = np.zeros(n_bins + 1, np.int64)
    np.cumsum(counts, out=starts[1:])
    slot_pos = np.arange(n_bins)[:, None] * epb + np.arange(epb)[None, :]
    for b in range(n_bins):
        e_orig[b * epb: b * epb + counts[b]] = order[starts[b]:starts[b + 1]]
    del slot_pos

    valid = e_orig >= 0
    src_slot = np.zeros(tot, np.int64)
    src_slot[valid] = src_s[e_orig[valid]]
    dst_loc = np.full(tot, -1.0, np.float32)      # local id in block, -1 = pad
    dst_flat = np.zeros(tot, np.int64)            # slot within core [0, npc)
    dst_v = dst_s[e_orig[valid]]
    dst_loc[valid] = (dst_v % 128).astype(np.float32)
    dst_flat[valid] = dst_v % npc
    edge_h_slot = np.zeros((tot, DE), np.float32)
    edge_h_slot[valid] = edge_h[e_orig[valid]]

    # node pair table (f16, zero-padded rows): [npad//2, 4*DN]
    nodeh_pad = np.zeros((npad, 2 * DN), np.float16)
    nodeh_pad[:, :DN] = node_h_p.astype(np.float16)
    nodeh_pairs = nodeh_pad.reshape(npad // 2, 4 * DN)

    # per-core tensors
    epc = bpc * epb
    in_maps = []
    W = {k: np.asarray(inputs[k], np.float32) for k in
         ("W_n1", "W_n2", "W_e1", "W_e2", "W_c", "W_ue")}
    b_ = {k: np.asarray(inputs[k], np.float32) for k in
          ("b_n1", "b_n2", "b_e1", "b_e2", "b_c")}
    # iota tiled for one group's 4 subtiles: [128, 512]
    iota_g = np.tile(np.arange(128, dtype=np.float16)[None, :], (128, GROUP // 128))
    for c in range(n_cores):
        sl = slice(c * bpc * epb, (c + 1) * bpc * epb)
        srcpair = _wrap_idx((src_slot[sl] // 2).astype(np.int16))      # [128, epc//16]
        par = (src_slot[sl] % 2).astype(np.float16)
        srcpar = np.stack([par, (1.0 - par).astype(np.float16)])       # [2, epc]
        dstflat = _wrap_idx(dst_flat[sl].astype(np.int16))             # [128, epc//16]
        dstloc = dst_loc[sl].astype(np.float16)
        dstloc_t = dstloc.reshape(bpc * epb // 128, 128).T             # [128, epc//128]
        in_maps.append({
            "nodeh_pairs": nodeh_pairs,
            "node_blk": node_h_p[c * npc:(c + 1) * npc].copy(),
            "edge_hT": np.ascontiguousarray(edge_h_slot[sl].astype(np.float16).T),
            "srcpair": np.ascontiguousarray(srcpair),
            "srcpar": np.ascontiguousarray(srcpar),
            "dstflat": np.ascontiguousarray(dstflat),
            "dstloc": np.ascontiguousarray(dstloc_t),
            "iota_g": iota_g,
            "W_n1": W["W_n1"].astype(np.float16),
            "W_n2": W["W_n2"].astype(np.float16),
            "W_e1": W["W_e1"].astype(np.float16),
            "W_e2": W["W_e2"].astype(np.float16),
            "W_c": W["W_c"].astype(np.float16),
            "W_ue02": ((1.0 - EMA) * W["W_ue"]).astype(np.float16),
            "b_n1": b_["b_n1"].reshape(HID, 1),
            "b_n2": b_["b_n2"].reshape(HID, 1),
            "b_e1": b_["b_e1"].reshape(HID, 1),
            "b_e2": b_["b_e2"].reshape(HID, 1),
            "b_c": b_["b_c"].reshape(DN, 1),
        })
    aux = dict(perm=perm, inv=inv, e_orig=e_orig, npc=npc, npad=npad,
               n_nodes=n_nodes, n_edges=n_edges)
    return in_maps, aux


# ---------------------------------------------------------------- device build

def _build(n_cores=N_CORES, bpc=BPC, epb=EPB):
    npc = bpc * 128
    npad = n_cores * npc
    epc = bpc * epb
    n_groups = epb // GROUP
    nc = bacc.Bacc("TRN2", target_bir_lowering=False, debug=False,
                   num_devices=n_cores, num_swdge_queues=2)

    def inp(name, shape, dt):
        return nc.dram_tensor(name, shape, dt, kind="ExternalInput").ap()

    nodeh_pairs = inp("nodeh_pairs", [npad // 2, 4 * DN], F16)
    node_blk = inp("node_blk", [npc, DN], F32)
    edge_hT = inp("edge_hT", [DE, epc], F16)
    srcpair = inp("srcpair", [128, epc // 16], I16)
    srcpar = inp("srcpar", [2, epc], F16)
    dstflat = inp("dstflat", [128, epc // 16], I16)
    dstloc = inp("dstloc", [128, epc // 128], F16)
    iota_g = inp("iota_g", [128, GROUP], F16)
    W_n1 = inp("W_n1", [DN, HID], F16)
    W_n2 = inp("W_n2", [HID, HID], F16)
    W_e1 = inp("W_e1", [DE, HID], F16)
    W_e2 = inp("W_e2", [HID, HID], F16)
    W_c = inp("W_c", [HID, DN], F16)
    W_ue02 = inp("W_ue02", [DN, DE], F16)
    b_n1 = inp("b_n1", [HID, 1], F32)
    b_n2 = inp("b_n2", [HID, 1], F32)
    b_e1 = inp("b_e1", [HID, 1], F32)
    b_e2 = inp("b_e2", [HID, 1], F32)
    b_c = inp("b_c", [DN, 1], F32)

    h_new_out = nc.dram_tensor("h_new", [npc, DN], F32, kind="ExternalOutput").ap()
    e_newT_out = nc.dram_tensor("e_newT", [DE, epc], F32, kind="ExternalOutput").ap()

    AF = mybir.ActivationFunctionType
    OP = mybir.AluOpType

    with tile.TileContext(nc) as tc:
        with tc.tile_pool(name="const", bufs=1) as cp, \
             tc.tile_pool(name="dram", bufs=1, space="DRAM") as dr:
            # persistent constants
            wn1 = cp.tile([DN, HID], F16); nc.sync.dma_start(out=wn1[:], in_=W_n1[:])
            wn2 = cp.tile([HID, HID], F16); nc.sync.dma_start(out=wn2[:], in_=W_n2[:])
            we1 = cp.tile([DE, HID], F16); nc.sync.dma_start(out=we1[:], in_=W_e1[:])
            we2 = cp.tile([HID, HID], F16); nc.sync.dma_start(out=we2[:], in_=W_e2[:])
            wc = cp.tile([HID, DN], F16); nc.sync.dma_start(out=wc[:], in_=W_c[:])
            wue = cp.tile([DN, DE], F16); nc.sync.dma_start(out=wue[:], in_=W_ue02[:])
            bn1 = cp.tile([HID, 1], F32); nc.sync.dma_start(out=bn1[:], in_=b_n1[:])
            bn2 = cp.tile([HID, 1], F32); nc.sync.dma_start(out=bn2[:], in_=b_n2[:])
            be1 = cp.tile([HID, 1], F32); nc.sync.dma_start(out=be1[:], in_=b_e1[:])
            be2 = cp.tile([HID, 1], F32); nc.sync.dma_start(out=be2[:], in_=b_e2[:])
            bc = cp.tile([DN, 1], F32); nc.sync.dma_start(out=bc[:], in_=b_c[:])
            iota = cp.tile([128, GROUP], F16); nc.sync.dma_start(out=iota[:], in_=iota_g[:])
            ident = cp.tile([128, 128], F16)
            make_identity(nc, ident[:])
            ones1 = cp.tile([1, DN], F16)
            nc.any.memset(ones1[:], 1.0)
            # core-local padded h table for phase-B dst gather:
            # [128, npc] f16; node slot n -> partition n%128, cols (n//128)*128..+64
            ltab = cp.tile([128, npc], F16)
            nc.any.memset(ltab[:], 0.0)

            ag_in = dr.tile([npc, 2 * DN], F16)
            hfull = dr.tile([npad, 2 * DN], F16, addr_space="Shared")

            # ---------------- phase A
            with tc.tile_pool(name="sbA", bufs=2) as sb, \
                 tc.tile_pool(name="psA", bufs=2, space="PSUM") as ps, \
                 tc.tile_pool(name="psA1", bufs=1, space="PSUM") as ps1, \
                 tc.tile_pool(name="psacc", bufs=1, space="PSUM") as psacc:
                for b in range(bpc):
                    esl = slice(b * epb, (b + 1) * epb)
                    sp_sb = sb.tile([128, epb // 16], I16)
                    nc.sync.dma_start(out=sp_sb[:], in_=srcpair[:, esl.start // 16: esl.stop // 16])
                    par_sb = sb.tile([1, epb], F16, tag="par")
                    nc.sync.dma_start(out=par_sb[:], in_=srcpar[0:1, esl])
                    inv_sb = sb.tile([1, epb], F16, tag="inv")
                    nc.sync.dma_start(out=inv_sb[:], in_=srcpar[1:2, esl])
                    dl_sb = sb.tile([128, epb // 128], F16)
                    nc.sync.dma_start(out=dl_sb[:], in_=dstloc[:, b * (epb // 128):(b + 1) * (epb // 128)])
                    eh_sb = sb.tile([DE, epb], F16)
                    nc.sync.dma_start(out=eh_sb[:], in_=edge_hT[:, esl])
                    nb_sb = sb.tile([128, DN], F32)
                    nc.sync.dma_start(out=nb_sb[:], in_=node_blk[b * 128:(b + 1) * 128, :])

                    # exact parity select masks for the whole block
                    mpar = sb.tile([DN, epb], F16, tag="mpar")
                    nc.gpsimd.partition_broadcast(mpar[:], par_sb[:])
                    minv = sb.tile([DN, epb], F16, tag="minv")
                    nc.gpsimd.partition_broadcast(minv[:], inv_sb[:])
                    hsrcT = sb.tile([DN, epb], F16)
                    for g in range(n_groups):
                        gsl = slice(g * GROUP, (g + 1) * GROUP)
                        gsrc = sb.tile([128, 2, GROUP], F16, tag="gsrc")
                        nc.gpsimd.dma_gather(
                            out_ap=gsrc[:], in_ap=nodeh_pairs[:],
                            idxs_ap=sp_sb[:, g * (GROUP // 16):(g + 1) * (GROUP // 16)],
                            num_idxs=GROUP, num_idxs_reg=GROUP, elem_size=4 * DN,
                            transpose=True, queue_num=g % 2)
                        t1 = sb.tile([DN, GROUP], F16, tag="selt1")
                        nc.vector.tensor_tensor(out=t1[:], in0=gsrc[0:DN, 0, :],
                                                in1=minv[:, gsl], op=OP.mult)
                        t2 = sb.tile([DN, GROUP], F16, tag="selt2")
                        nc.vector.tensor_tensor(out=t2[:], in0=gsrc[0:DN, 1, :],
                                                in1=mpar[:, gsl], op=OP.mult)
                        nc.vector.tensor_add(out=hsrcT[:, gsl], in0=t1[:], in1=t2[:])

                    hacc = psacc.tile([128, DN], F32, tag="acc")
                    for g in range(n_groups):
                        gsl = slice(g * GROUP, (g + 1) * GROUP)
                        a1ps = ps.tile([HID, GROUP], F32, tag="mlp1")
                        nc.tensor.matmul(out=a1ps[:], lhsT=wn1[:], rhs=hsrcT[:, gsl],
                                         start=True, stop=True)
                        a1 = sb.tile([HID, GROUP], F16, tag="a1")
                        nc.scalar.activation(out=a1[:], in_=a1ps[:], func=AF.Relu,
                                             bias=bn1[:, :1], scale=1.0)
                        m1ps = ps1.tile([HID, GROUP], F32, tag="mlp2")
                        nc.tensor.matmul(out=m1ps[:], lhsT=wn2[:], rhs=a1[:],
                                         start=True, stop=True)
                        m1 = sb.tile([HID, GROUP], F16, tag="m1")
                        nc.scalar.activation(out=m1[:], in_=m1ps[:], func=AF.Identity,
                                             bias=bn2[:, :1], scale=1.0)
                        a2ps = ps.tile([HID, GROUP], F32, tag="mlp1")
                        nc.tensor.matmul(out=a2ps[:], lhsT=we1[:], rhs=eh_sb[:, gsl],
                                         start=True, stop=True)
                        a2 = sb.tile([HID, GROUP], F16, tag="a2")
                        nc.scalar.activation(out=a2[:], in_=a2ps[:], func=AF.Relu,
                                             bias=be1[:, :1], scale=1.0)
                        m2ps = ps1.tile([HID, GROUP], F32, tag="mlp2")
                        nc.tensor.matmul(out=m2ps[:], lhsT=we2[:], rhs=a2[:],
                                         start=True, stop=True)
                        m2 = sb.tile([HID, GROUP], F16, tag="m2")
                        nc.scalar.activation(out=m2[:], in_=m2ps[:], func=AF.Identity,
                                             bias=be2[:, :1], scale=1.0)
                        mm = sb.tile([HID, GROUP], F16, tag="mm")
                        nc.vector.tensor_tensor(out=mm[:], in0=m1[:], in1=m2[:],
                                                op=OP.mult)
                        zps = ps1.tile([DN, GROUP], F32, tag="z")
                        nc.tensor.matmul(out=zps[:], lhsT=wc[:], rhs=mm[:],
                                         start=True, stop=True)
                        mT = sb.tile([DN, GROUP], F16, tag="mT")
                        nc.scalar.activation(out=mT[:], in_=zps[:], func=AF.Tanh,
                                             bias=bc[:, :1], scale=1.0)
                        # onehot for the 4 subtiles of this group
                        oh = sb.tile([128, GROUP], F16, tag="oh")
                        t0 = g * (GROUP // 128)
                        nc.vector.tensor_tensor(
                            out=oh[:].rearrange("p (s j) -> p s j", j=128),
                            in0=dl_sb[:, t0:t0 + GROUP // 128].unsqueeze(2).to_broadcast(
                                [128, GROUP // 128, 128]),
                            in1=iota[:].rearrange("p (s j) -> p s j", j=128),
                            op=OP.is_equal)
                        for s in range(GROUP // 128):
                            mem_ps = psacc.tile([128, DN], F16, tag="mem")
                            nc.tensor.transpose(out=mem_ps[:],
                                                in_=mT[:, s * 128:(s + 1) * 128],
                                                identity=ident[0:DN, 0:DN])
                            mem = sb.tile([128, DN], F16, tag="mem_sb")
                            nc.vector.tensor_copy(out=mem[:], in_=mem_ps[:])
                            t = g * (GROUP // 128) + s
                            nc.tensor.matmul(out=hacc[:],
                                             lhsT=oh[:, s * 128:(s + 1) * 128],
                                             rhs=mem[:],
                                             start=(t == 0), stop=(t == epb // 128 - 1),
                                             skip_group_check=True)
                    hblk = sb.tile([128, DN], F32, tag="hblk")
                    nc.vector.tensor_add(out=hblk[:], in0=hacc[:], in1=nb_sb[:])
                    nc.sync.dma_start(out=h_new_out[b * 128:(b + 1) * 128, :], in_=hblk[:])
                    h16p = sb.tile([128, 2 * DN], F16, tag="h16p")
                    nc.any.memset(h16p[:], 0.0)
                    nc.vector.tensor_copy(out=h16p[:, 0:DN], in_=hblk[:])
                    nc.sync.dma_start(out=ag_in[b * 128:(b + 1) * 128, :], in_=h16p[:])
                    # local padded table columns [b*128, (b+1)*128)
                    nc.vector.tensor_copy(out=ltab[:, b * 128:b * 128 + DN],
                                          in_=h16p[:, 0:DN])

            # ---------------- allgather
            nc.gpsimd.collective_compute(
                "AllGather", mybir.AluOpType.bypass,
                replica_groups=[list(range(n_cores))],
                ins=[ag_in.opt()], outs=[hfull.opt()])

            # ---------------- phase B
            with tc.tile_pool(name="sbB", bufs=2) as sb, \
                 tc.tile_pool(name="psB", bufs=2, space="PSUM") as ps:
                for b in range(bpc):
                    esl = slice(b * epb, (b + 1) * epb)
                    sp_sb = sb.tile([128, epb // 16], I16, tag="spB")
                    nc.sync.dma_start(out=sp_sb[:], in_=srcpair[:, esl.start // 16: esl.stop // 16])
                    par_sb = sb.tile([1, epb], F16, tag="parB")
                    nc.sync.dma_start(out=par_sb[:], in_=srcpar[0:1, esl])
                    inv_sb = sb.tile([1, epb], F16, tag="invB")
                    nc.sync.dma_start(out=inv_sb[:], in_=srcpar[1:2, esl])
                    df_sb = sb.tile([128, epb // 16], I16, tag="dfB")
                    nc.sync.dma_start(out=df_sb[:], in_=dstflat[:, esl.start // 16: esl.stop // 16])
                    eh_sb = sb.tile([DE, epb], F16, tag="ehB")
                    nc.sync.dma_start(out=eh_sb[:], in_=edge_hT[:, esl])

                    mpar = sb.tile([DN, epb], F16, tag="mparB")
                    nc.gpsimd.partition_broadcast(mpar[:], par_sb[:])
                    minv = sb.tile([DN, epb], F16, tag="minvB")
                    nc.gpsimd.partition_broadcast(minv[:], inv_sb[:])
                    uT = sb.tile([DN, epb], F16, tag="uB")
                    for g in range(n_groups):
                        gsl = slice(g * GROUP, (g + 1) * GROUP)
                        isl = slice(g * (GROUP // 16), (g + 1) * (GROUP // 16))
                        gs = sb.tile([128, 2, GROUP], F16, tag="gsB")
                        nc.gpsimd.dma_gather(
                            out_ap=gs[:],
                            in_ap=hfull[:].rearrange("(a b) c -> a (b c)", b=2),
                            idxs_ap=sp_sb[:, isl],
                            num_idxs=GROUP, num_idxs_reg=GROUP, elem_size=4 * DN,
                            transpose=True, queue_num=0)
                        gd = sb.tile([128, 1, GROUP], F16, tag="gdB")
                        nc.gpsimd.dma_gather(
                            out_ap=gd[:], in_ap=ltab[:], idxs_ap=df_sb[:, isl],
                            num_idxs=GROUP, num_idxs_reg=GROUP, elem_size=128,
                            transpose=True, queue_num=1,
                            sbuf_tokens_per_rank=128, sbuf_free_dim_per_rank=256)
                        t1 = sb.tile([DN, GROUP], F16, tag="selt1B")
                        nc.vector.tensor_tensor(out=t1[:], in0=gs[0:DN, 0, :],
                                                in1=minv[:, gsl], op=OP.mult)
                        t2 = sb.tile([DN, GROUP], F16, tag="selt2B")
                        nc.vector.tensor_tensor(out=t2[:], in0=gs[0:DN, 1, :],
                                                in1=mpar[:, gsl], op=OP.mult)
                        hsT = sb.tile([DN, GROUP], F16, tag="hsB")
                        nc.vector.tensor_add(out=hsT[:], in0=t1[:], in1=t2[:])
                        nc.vector.tensor_tensor(out=uT[:, gsl], in0=hsT[:],
                                                in1=gd[0:DN, 0, :], op=OP.mult)
                    for g in range(n_groups):
                        gsl = slice(g * GROUP, (g + 1) * GROUP)
                        xps = ps.tile([DE, GROUP], F32, tag="xB")
                        nc.tensor.matmul(out=xps[:], lhsT=wue[:], rhs=uT[:, gsl],
                                         start=True, stop=True)
                        eh32 = sb.tile([DE, GROUP], F32, tag="eh32B")
                        nc.vector.tensor_copy(out=eh32[:], in_=eh_sb[:, gsl])
                        eout = sb.tile([DE, GROUP], F32, tag="eoutB")
                        nc.vector.scalar_tensor_tensor(
                            out=eout[:], in0=eh32[:], scalar=float(EMA),
                            in1=xps[:], op0=OP.mult, op1=OP.add)
                        nc.sync.dma_start(
                            out=e_newT_out[:, b * epb + g * GROUP: b * epb + (g + 1) * GROUP],
                            in_=eout[:])
    nc.compile()
    return nc


# ---------------------------------------------------------------- entry points

_NC_CACHE = {}


def _get_nc(n_cores=N_CORES, bpc=BPC, epb=EPB):
    key = (n_cores, bpc, epb)
    if key not in _NC_CACHE:
        _NC_CACHE[key] = _build(n_cores, bpc, epb)
    return _NC_CACHE[key]


def _run(inputs, trace=False, n_cores=N_CORES, bpc=BPC, epb=EPB,
         n_nodes=N_NODES, n_edges=N_EDGES, trace_kwargs=None):
    from concourse.bass_utils import run_bass_kernel_spmd
    in_maps, aux = _prep(inputs, n_cores, bpc, epb, n_nodes, n_edges)
    nc = _get_nc(n_cores, bpc, epb)
    kw = {}
    if trace:
        kw = dict(trace=True, **(trace_kwargs or {}))
    res = run_bass_kernel_spmd(nc, in_maps, list(range(n_cores)), **kw)

    npc, npad = aux["npc"], aux["npad"]
    h_cat = np.concatenate([res.results[c]["h_new"] for c in range(n_cores)], axis=0)
    h_new = np.empty((n_nodes, DN), np.float32)
    perm = aux["perm"]
    real = perm < n_nodes
    h_new[perm[real]] = h_cat[real]

    eT_cat = np.concatenate([res.results[c]["e_newT"] for c in range(n_cores)], axis=1)
    e_slots = eT_cat.T                      # [n_bins*epb, DE]
    e_new = np.empty((n_edges, DE), np.float32)
    e_orig = aux["e_orig"]
    valid = e_orig >= 0
    e_new[e_orig[valid]] = e_slots[valid]
    return (h_new, e_new), res


def kernel(**inputs):
    (h_new, e_new), _ = _run(inputs)
    return h_new, e_new
